# revision 3
# baseline (speedup 1.0000x reference)
"""DIN-style attention + Dice + MLP kernel for 8 trn2 NeuronCores.

Math (reference):
    q = query[gather_idx]                  # [T, 64]
    p = flat outer(x, q)                   # [T, 4096]
    h = [x, p, q]                          # [T, 4224]
    z = h @ W1 + b1                        # [T, 256]
    z = Dice(z)  (batch-global mean/var over T, ddof=1, sigmoid gate)
    out = z @ W2 + b2                      # [T, 1]

Key factorization: for t in group b (gather_idx[t] == b),
    z[t] = x_aug[t] @ D_b,   x_aug = [x, 1],
    D_b[j', a] = (j'<64): W1x[j',a] + sum_j query[b,j] W1p[j',j,a]
                 (j'=64): sum_j query[b,j] W1q[j,a] + b1[a]
so the [T,4096] outer-product features are never materialized; the dense
[T,4224]x[4224,256] matmul (137 GFLOP) becomes ~5 GFLOP of small matmuls.

Sharding: timesteps are grouped by gather value; the 512 groups are dealt
round-robin by descending size to 8 cores x 64 slots, so slot s has the same
padded width G_s on every core (one SPMD graph). Padded columns have x=0 and
mask=0 so their z is exactly 0 and global Dice sums (all-reduced across
cores, 2KB AllReduce) stay exact with T hardcoded as the real count.
"""

import numpy as np
import ml_dtypes

NCORE = 8
LAST_EXEC_NS = None
LAST_RESULT = None


def _build_and_run(x, query, gather_idx, W1, b1, alpha, W2, b2):
    import concourse.bass as bass
    import concourse.tile as tile
    from concourse import bacc, mybir, bass_utils
    from contextlib import ExitStack

    f32 = mybir.dt.float32
    bf16 = mybir.dt.bfloat16
    AF = mybir.ActivationFunctionType
    bf_np = ml_dtypes.bfloat16

    T, D = x.shape
    B = query.shape[0]
    A = W1.shape[1]
    EPS = 1e-9
    SLOTS = B // NCORE
    assert W1.shape[0] == D + D * D + D and B % NCORE == 0

    # ---- host-side sharding / layout ------------------------------------
    counts = np.bincount(gather_idx, minlength=B)
    order = np.argsort(-counts, kind="stable")  # groups by count desc
    Gs = []
    for s in range(SLOTS):
        m = int(counts[order[s * NCORE:(s + 1) * NCORE]].max())
        Gs.append(max(8, -(-m // 8) * 8))
    col_start = np.concatenate([[0], np.cumsum(Gs)]).astype(np.int64)
    Ncol = int(col_start[-1])
    assert max(Gs) <= 512, f"group too large: {max(Gs)}"

    sort_t = np.argsort(gather_idx, kind="stable")
    gstart = np.concatenate([[0], np.cumsum(counts)]).astype(np.int64)

    xT = np.ascontiguousarray(x.T.astype(np.float32))
    Xc = np.zeros((NCORE, D + 1, Ncol), np.float32)
    Qc = np.zeros((NCORE, D + 1, SLOTS), np.float32)
    idx_map = np.zeros((NCORE, Ncol), np.int64)
    valid = np.zeros((NCORE, Ncol), bool)
    for c in range(NCORE):
        for s in range(SLOTS):
            g = int(order[s * NCORE + c])
            n = int(counts[g])
            c0 = int(col_start[s])
            ts = sort_t[gstart[g]:gstart[g] + n]
            Xc[c, :D, c0:c0 + n] = xT[:, ts]
            Xc[c, D, c0:c0 + n] = 1.0
            idx_map[c, c0:c0 + n] = ts
            valid[c, c0:c0 + n] = True
            Qc[c, :D, s] = query[g]
            Qc[c, D, s] = 1.0
    Xc16 = Xc.astype(bf_np)
    Qc16 = Qc.astype(bf_np)

    W1x = W1[:D]
    W1p = W1[D:D + D * D].reshape(D, D, A)  # [i, j, a]
    W1q = W1[D + D * D:]
    Waug = np.zeros((D + 1, D + 1, A), np.float32)  # [j, i', a]
    Waug[:D, :D, :] = np.transpose(W1p, (1, 0, 2))
    Waug[:D, D, :] = W1q
    Waug[D, :D, :] = W1x
    Waug[D, D, :] = b1
    Waug16 = Waug.astype(bf_np)

    al = float(np.asarray(alpha).reshape(-1)[0])
    alpha_nz = al != 0.0
    w2v = np.asarray(W2, np.float32).reshape(-1)
    w_y = w2v * (1.0 - al)
    w_z = w2v * al
    AH = A // 2  # 128
    wdot = np.stack([w_y[:AH], w_y[AH:], w_z[:AH], w_z[AH:]], axis=1)
    wdot16 = wdot.astype(bf_np)
    b2v = np.asarray(b2, np.float32).reshape(1, 1)

    in_maps = [
        {"xc": Xc16[c], "qc": Qc16[c], "waug": Waug16, "wdot": wdot16,
         "b2": b2v}
        for c in range(NCORE)
    ]

    # ---- device graph ----------------------------------------------------
    nc = bacc.Bacc("TRN2", target_bir_lowering=False, debug=False,
                   num_devices=NCORE)
    xd = nc.dram_tensor("xc", [D + 1, Ncol], bf16, kind="ExternalInput")
    qd = nc.dram_tensor("qc", [D + 1, SLOTS], bf16, kind="ExternalInput")
    wd = nc.dram_tensor("waug", [D + 1, D + 1, A], bf16, kind="ExternalInput")
    wdotd = nc.dram_tensor("wdot", [AH, 4], bf16, kind="ExternalInput")
    b2d = nc.dram_tensor("b2", [1, 1], f32, kind="ExternalInput")
    outd = nc.dram_tensor("out", [1, Ncol], f32, kind="ExternalOutput")

    CH = 512
    nch = -(-Ncol // CH)
    ABLK = 8  # a-columns per C-stage psum tile

    with tile.TileContext(nc) as tc, ExitStack() as ctx:
        consts = ctx.enter_context(tc.tile_pool(name="consts", bufs=1))
        waug_sb = consts.tile([D + 1, D + 1, A], bf16, tag="waug")
        qc_sb = consts.tile([D + 1, SLOTS], bf16, tag="qc")
        x_sb = consts.tile([D + 1, Ncol], bf16, tag="x")
        wdot_sb = consts.tile([AH, 4], bf16, tag="wdot")
        b2_sb = consts.tile([1, 1], f32, tag="b2")
        eps_sb = consts.tile([AH, 1], f32, tag="eps")
        dpp = consts.tile([D + 1, A, SLOTS], bf16, tag="dpp")
        z_sb = consts.tile([AH, 2, Ncol], bf16, tag="z")
        out_sb = consts.tile([1, Ncol], f32, tag="outsb")
        stats = consts.tile([AH, 2, nch, 6], f32, tag="stats")
        mv = consts.tile([AH, 2, 2], f32, tag="mv")
        cc_sb = consts.tile([AH, 8], f32, tag="cc")
        fin = consts.tile([AH, 2, 4], f32, tag="fin")

        nc.sync.dma_start(out=waug_sb, in_=wd.ap())
        nc.sync.dma_start(out=qc_sb, in_=qd.ap())
        nc.sync.dma_start(out=x_sb, in_=xd.ap())
        nc.sync.dma_start(out=wdot_sb, in_=wdotd.ap())
        nc.sync.dma_start(out=b2_sb, in_=b2d.ap())
        nc.vector.memset(eps_sb, EPS)

        # C-stage: per-slot MLP matrices D_b, built as 256 small matmuls
        # out[i', b] = sum_j Waug[j, i', a] * q_aug[j, b], laid [i', a, b]
        with tc.tile_pool(name="psC", bufs=4, space="PSUM") as psC:
            for blk in range(A // ABLK):
                ps = psC.tile([D + 1, ABLK, SLOTS], f32, tag="c")
                for k in range(ABLK):
                    a = blk * ABLK + k
                    nc.tensor.matmul(out=ps[:, k, :], lhsT=waug_sb[:, :, a],
                                     rhs=qc_sb, start=True, stop=True)
                nc.scalar.copy(out=dpp[:, blk * ABLK:(blk + 1) * ABLK, :],
                               in_=ps)

        # Group stage: z^T[a, t] for each slot, both A-halves
        with tc.tile_pool(name="psG", bufs=6, space="PSUM") as psG:
            for s in range(SLOTS):
                c0, w = int(col_start[s]), Gs[s]
                for h in range(2):
                    ps = psG.tile([AH, CH], f32, tag="g")
                    nc.tensor.matmul(out=ps[:, :w],
                                     lhsT=dpp[:, h * AH:(h + 1) * AH, s],
                                     rhs=x_sb[:, c0:c0 + w],
                                     start=True, stop=True)
                    nc.vector.tensor_copy(out=z_sb[:, h, c0:c0 + w],
                                          in_=ps[:, :w])

        # Batch stats (pad columns are exact zeros; sums unaffected)
        for h in range(2):
            for ci in range(nch):
                c0 = ci * CH
                w = min(CH, Ncol - c0)
                nc.vector.bn_stats(out=stats[:, h, ci, :],
                                   in_=z_sb[:, h, c0:c0 + w])
            nc.vector.bn_aggr(out=mv[:, h, :], in_=stats[:, h, :, :])
        nc.vector.memset(cc_sb, 0.0)
        for h in range(2):
            mean = mv[:, h, 0:1]
            var = mv[:, h, 1:2]
            tmp = fin[:, h, 3:4]
            nc.vector.tensor_scalar_mul(cc_sb[:, 2 * h:2 * h + 1], mean,
                                        float(Ncol))
            nc.vector.tensor_mul(tmp, mean, mean)
            nc.vector.tensor_add(tmp, tmp, var)
            nc.vector.tensor_scalar_mul(cc_sb[:, 2 * h + 1:2 * h + 2], tmp,
                                        float(Ncol))

        # Cross-core AllReduce of [sum, sumsq] per output unit (2KB)
        with tc.tile_pool(name="dram", bufs=1, space="DRAM") as dram:
            cc_in = dram.tile([AH, 8], f32, tag="ccin")
            cc_out = dram.tile([AH, 8], f32, tag="ccout")
            nc.gpsimd.dma_start(out=cc_in[:], in_=cc_sb[:])
            nc.gpsimd.collective_compute(
                "AllReduce", mybir.AluOpType.add,
                replica_groups=[list(range(NCORE))],
                ins=[cc_in.opt()], outs=[cc_out.opt()])
            nc.gpsimd.dma_start(out=cc_sb[:], in_=cc_out[:])

        # Finalize: mean, rstd, -mean*rstd per half (ddof=1 over real T)
        Tf = float(T)
        for h in range(2):
            S1 = cc_sb[:, 2 * h:2 * h + 1]
            S2 = cc_sb[:, 2 * h + 1:2 * h + 2]
            mean = fin[:, h, 0:1]
            rstd = fin[:, h, 1:2]
            nb = fin[:, h, 2:3]
            tmp = fin[:, h, 3:4]
            nc.vector.tensor_scalar_mul(mean, S1, 1.0 / Tf)
            nc.vector.tensor_mul(tmp, S1, mean)
            nc.vector.tensor_sub(tmp, S2, tmp)
            nc.vector.tensor_scalar_mul(tmp, tmp, 1.0 / (Tf - 1.0))
            nc.scalar.activation(out=tmp, in_=tmp, func=AF.Sqrt, bias=eps_sb,
                                 scale=1.0)
            nc.vector.reciprocal(rstd, tmp)
            nc.vector.tensor_mul(nb, mean, rstd)
            nc.vector.tensor_scalar_mul(nb, nb, -1.0)

        # Tail: gate + weighted column-dot via PE, + b2
        with tc.tile_pool(name="tails", bufs=4) as tails, \
                tc.tile_pool(name="psD", bufs=4, space="PSUM") as psD:
            total_mm = 4 if alpha_nz else 2
            for ci in range(nch):
                c0 = ci * CH
                w = min(CH, Ncol - c0)
                ps = psD.tile([1, CH], f32, tag="d")
                nmm = 0
                for h in range(2):
                    s_t = tails.tile([AH, CH], bf16, tag="s")
                    nc.scalar.activation(out=s_t[:, :w],
                                         in_=z_sb[:, h, c0:c0 + w],
                                         func=AF.Sigmoid,
                                         bias=fin[:, h, 2:3],
                                         scale=fin[:, h, 1:2])
                    y_t = tails.tile([AH, CH], bf16, tag="y")
                    nc.vector.tensor_mul(y_t[:, :w], z_sb[:, h, c0:c0 + w],
                                         s_t[:, :w])
                    nc.tensor.matmul(out=ps[:, :w], lhsT=wdot_sb[:, h:h + 1],
                                     rhs=y_t[:, :w], start=(nmm == 0),
                                     stop=(nmm == total_mm - 1))
                    nmm += 1
                if alpha_nz:
                    for h in range(2):
                        nc.tensor.matmul(out=ps[:, :w],
                                         lhsT=wdot_sb[:, 2 + h:3 + h],
                                         rhs=z_sb[:, h, c0:c0 + w],
                                         start=False,
                                         stop=(nmm == total_mm - 1))
                        nmm += 1
                nc.scalar.activation(out=out_sb[:, c0:c0 + w], in_=ps[:, :w],
                                     func=AF.Identity, bias=b2_sb[0:1, 0:1],
                                     scale=1.0)
            nc.sync.dma_start(out=outd.ap(), in_=out_sb)

    nc.compile()
    import os
    trace = bool(os.environ.get("DIN_TRACE"))
    res = bass_utils.run_bass_kernel_spmd(nc, in_maps,
                                          core_ids=list(range(NCORE)),
                                          trace=trace,
                                          trace_cores=list(range(NCORE))
                                          if trace else None)
    global LAST_EXEC_NS, LAST_RESULT
    LAST_EXEC_NS = res.exec_time_ns
    LAST_RESULT = res

    full = np.zeros((T, 1), np.float32)
    for c in range(NCORE):
        o = np.asarray(res.results[c]["out"], np.float32).reshape(-1)
        full[idx_map[c][valid[c]], 0] = o[valid[c]]
    return full


def kernel(x, query, gather_idx, W1, b1, alpha, W2, b2):
    return _build_and_run(
        np.asarray(x, np.float32), np.asarray(query, np.float32),
        np.asarray(gather_idx), np.asarray(W1, np.float32),
        np.asarray(b1, np.float32), np.asarray(alpha, np.float32),
        np.asarray(W2, np.float32), np.asarray(b2, np.float32))


# revision 8
# speedup vs baseline: 1.3234x; 1.3234x over previous
"""DIN-style attention + Dice + MLP kernel for 8 trn2 NeuronCores.

Math (reference):
    q = query[gather_idx]                  # [T, 64]
    p = flat outer(x, q)                   # [T, 4096]
    h = [x, p, q]                          # [T, 4224]
    z = h @ W1 + b1                        # [T, 256]
    z = Dice(z)  (batch-global mean/var over T, ddof=1, sigmoid gate)
    out = z @ W2 + b2                      # [T, 1]

Key factorization: for t in group b (gather_idx[t] == b),
    z[t] = x_aug[t] @ D_b,   x_aug = [x, mask],
    D_b[j', a] = (j'<64): W1x[j',a] + sum_j query[b,j] W1p[j',j,a]
                 (j'=64): sum_j query[b,j] W1q[j,a] + b1[a]
so the [T,4096] outer-product features are never materialized; the dense
[T,4224]x[4224,256] matmul (137 GFLOP) becomes ~5 GFLOP of small matmuls.

Sharding: timesteps are grouped by gather value; the 512 groups are dealt
round-robin by descending size to 8 cores x 64 slots, so slot s has the same
padded width G_s on every core (one SPMD graph). Padded columns have x=0 and
mask=0 so their z is exactly 0 and global Dice sums (AllGathered across
cores, 4KB) stay exact with T hardcoded as the real count.
"""

import numpy as np
import ml_dtypes

NCORE = 8
LAST_EXEC_NS = None
LAST_RESULT = None


def _build(x, query, gather_idx, W1, b1, alpha, W2, b2):
    import concourse.bass as bass
    import concourse.tile as tile
    from concourse import bacc, mybir, bass_utils
    from contextlib import ExitStack

    f32 = mybir.dt.float32
    bf16 = mybir.dt.bfloat16
    AF = mybir.ActivationFunctionType
    ALU = mybir.AluOpType
    bf_np = ml_dtypes.bfloat16

    T, D = x.shape
    B = query.shape[0]
    A = W1.shape[1]
    EPS = 1e-9
    SLOTS = B // NCORE
    assert W1.shape[0] == D + D * D + D and B % NCORE == 0

    # ---- host-side sharding / layout ------------------------------------
    counts = np.bincount(gather_idx, minlength=B)
    order = np.argsort(-counts, kind="stable")  # groups by count desc
    Gs = []
    for s in range(SLOTS):
        m = int(counts[order[s * NCORE:(s + 1) * NCORE]].max())
        Gs.append(max(8, -(-m // 8) * 8))
    col_start = np.concatenate([[0], np.cumsum(Gs)]).astype(np.int64)
    Ncol = int(col_start[-1])
    assert max(Gs) <= 512, f"group too large: {max(Gs)}"

    # pack slots into PSUM-bank-sized column ranges (<=512 fp32)
    packs = []  # (slot_lo, slot_hi) half-open
    lo = 0
    while lo < SLOTS:
        hi = lo + 1
        while hi < SLOTS and col_start[hi + 1] - col_start[lo] <= 512:
            hi += 1
        packs.append((lo, hi))
        lo = hi
    NP = len(packs)

    sort_t = np.argsort(gather_idx, kind="stable")
    gstart = np.concatenate([[0], np.cumsum(counts)]).astype(np.int64)

    xT = np.ascontiguousarray(x.T.astype(np.float32))
    Xc = np.zeros((NCORE, D + 1, Ncol), np.float32)
    Qc = np.zeros((NCORE, D + 1, SLOTS), np.float32)
    idx_map = np.zeros((NCORE, Ncol), np.int64)
    valid = np.zeros((NCORE, Ncol), bool)
    for c in range(NCORE):
        for s in range(SLOTS):
            g = int(order[s * NCORE + c])
            n = int(counts[g])
            c0 = int(col_start[s])
            ts = sort_t[gstart[g]:gstart[g] + n]
            Xc[c, :D, c0:c0 + n] = xT[:, ts]
            Xc[c, D, c0:c0 + n] = 1.0
            idx_map[c, c0:c0 + n] = ts
            valid[c, c0:c0 + n] = True
            Qc[c, :D, s] = query[g]
            Qc[c, D, s] = 1.0
    Xc16 = np.ascontiguousarray(Xc.astype(bf_np))
    Qc16 = np.ascontiguousarray(Qc.astype(bf_np))

    W1x = W1[:D]
    W1p = W1[D:D + D * D].reshape(D, D, A)  # [i, j, a]
    W1q = W1[D + D * D:]
    Waug = np.zeros((D + 1, D + 1, A), np.float32)  # [j, i', a]
    Waug[:D, :D, :] = np.transpose(W1p, (1, 0, 2))
    Waug[:D, D, :] = W1q
    Waug[D, :D, :] = W1x
    Waug[D, D, :] = b1
    Waug16 = np.ascontiguousarray(Waug.astype(bf_np))

    al = float(np.asarray(alpha).reshape(-1)[0])
    alpha_nz = al != 0.0
    b2f = float(np.asarray(b2).reshape(-1)[0])
    b2_nz = b2f != 0.0
    w2v = np.asarray(W2, np.float32).reshape(-1)
    w_y = w2v * (1.0 - al)
    w_z = w2v * al
    AH = A // 2  # 128
    wdot = np.stack([w_y[:AH], w_y[AH:], w_z[:AH], w_z[AH:]], axis=1)
    wdot16 = np.ascontiguousarray(wdot.astype(bf_np))
    b2v = np.asarray([[b2f]]).astype(bf_np)

    in_maps = [
        {"xc": Xc16[c], "qc": Qc16[c], "waug": Waug16, "wdot": wdot16,
         "b2": b2v}
        for c in range(NCORE)
    ]

    # ---- device graph ----------------------------------------------------
    nc = bacc.Bacc("TRN2", target_bir_lowering=False, debug=False,
                   num_devices=NCORE)
    xd = nc.dram_tensor("xc", [D + 1, Ncol], bf16, kind="ExternalInput")
    qd = nc.dram_tensor("qc", [D + 1, SLOTS], bf16, kind="ExternalInput")
    wd = nc.dram_tensor("waug", [D + 1, D + 1, A], bf16, kind="ExternalInput")
    wdotd = nc.dram_tensor("wdot", [AH, 4], bf16, kind="ExternalInput")
    b2d = nc.dram_tensor("b2", [1, 1], bf16, kind="ExternalInput")
    outd = nc.dram_tensor("out", [1, Ncol], f32, kind="ExternalOutput")

    ABLK = 8          # a-columns per C-stage psum tile
    WCHUNK = 32       # a-columns per waug DMA chunk
    TCH = 1024        # tail sigmoid/mul chunk
    nch_t = -(-Ncol // TCH)

    with tile.TileContext(nc) as tc, ExitStack() as ctx:
        consts = ctx.enter_context(tc.tile_pool(name="consts", bufs=1))
        waug_sb = consts.tile([D + 1, D + 1, A], bf16, tag="waug")
        qc_sb = consts.tile([D + 1, SLOTS], bf16, tag="qc")
        x_sb = consts.tile([D + 1, Ncol], bf16, tag="x")
        wdot_sb = consts.tile([AH, 4], bf16, tag="wdot")
        b2_sb = consts.tile([1, 1], bf16, tag="b2")
        ones_sb = consts.tile([1, 512], bf16, tag="ones")
        eps_sb = consts.tile([AH, 1], f32, tag="eps")
        warm_sb = consts.tile([AH, 1], f32, tag="warm")
        dpp = consts.tile([D + 1, A, SLOTS], bf16, tag="dpp")
        z_sb = consts.tile([AH, 2, Ncol], bf16, tag="z")
        out_sb = consts.tile([1, Ncol], f32, tag="outsb")
        stats = consts.tile([AH, 2, NP, 6], f32, tag="stats")
        mv = consts.tile([AH, 2, 2], f32, tag="mv")
        ccin_sb = consts.tile([AH, 8], f32, tag="cc")
        ccg_sb = consts.tile([AH, NCORE, 8], f32, tag="ccg")
        fin = consts.tile([AH, 2, 4], f32, tag="fin")

        # input DMAs; waug chunked along a so the C-stage starts early
        nc.sync.dma_start(out=qc_sb, in_=qd.ap())
        for q0 in range(0, A, WCHUNK):
            nc.sync.dma_start(out=waug_sb[:, :, q0:q0 + WCHUNK],
                              in_=wd.ap()[:, :, q0:q0 + WCHUNK])
        nc.sync.dma_start(out=x_sb, in_=xd.ap())
        nc.sync.dma_start(out=wdot_sb, in_=wdotd.ap())
        nc.sync.dma_start(out=b2_sb, in_=b2d.ap())
        nc.vector.memset(eps_sb, EPS)
        nc.vector.memset(ones_sb, 1.0)
        nc.vector.memset(warm_sb, 0.0)
        # pre-load ACT tables off the critical path
        nc.scalar.activation(out=warm_sb, in_=warm_sb, func=AF.Sigmoid)
        nc.scalar.activation(out=warm_sb, in_=warm_sb, func=AF.Identity,
                             bias=0.0, scale=1.0)

        # C-stage: per-slot MLP matrices D_b, built as 256 small matmuls
        # out[i', b] = sum_j Waug[j, i', a] * q_aug[j, b], laid [i', a, b]
        with tc.tile_pool(name="psC", bufs=6, space="PSUM") as psC:
            for blk in range(A // ABLK):
                ps = psC.tile([D + 1, ABLK, SLOTS], f32, tag="c")
                for k in range(ABLK):
                    a = blk * ABLK + k
                    nc.tensor.matmul(out=ps[:, k, :], lhsT=waug_sb[:, :, a],
                                     rhs=qc_sb, start=True, stop=True)
                nc.scalar.copy(out=dpp[:, blk * ABLK:(blk + 1) * ABLK, :],
                               in_=ps)

        # Group stage: z^T[a, t] per slot; slots packed into 512-col psum
        # tiles; evac on ACT fuses the Dice sum, DVE TTR fuses the sq-sum.
        with tc.tile_pool(name="psG", bufs=6, space="PSUM") as psG:
            for pi, (lo, hi) in enumerate(packs):
                p0 = int(col_start[lo])
                wsum = int(col_start[hi]) - p0
                for h in range(2):
                    ps = psG.tile([AH, 512], f32, tag="g")
                    for s in range(lo, hi):
                        c0 = int(col_start[s]) - p0
                        w = Gs[s]
                        nc.tensor.matmul(
                            out=ps[:, c0:c0 + w],
                            lhsT=dpp[:, h * AH:(h + 1) * AH, s],
                            rhs=x_sb[:, p0 + c0:p0 + c0 + w],
                            start=True, stop=True)
                    nc.scalar.copy(out=z_sb[:, h, p0:p0 + wsum],
                                   in_=ps[:, :wsum])
                    nc.vector.bn_stats(out=stats[:, h, pi, :],
                                       in_=z_sb[:, h, p0:p0 + wsum])

        # Pack [S1, S2] per half and exchange all cores' partials (4KB)
        nc.vector.memset(ccin_sb, 0.0)
        for h in range(2):
            nc.vector.bn_aggr(out=mv[:, h, :], in_=stats[:, h, :, :])
            mean = mv[:, h, 0:1]
            var = mv[:, h, 1:2]
            tmp = fin[:, h, 3:4]
            nc.vector.tensor_scalar_mul(ccin_sb[:, 2 * h:2 * h + 1], mean,
                                        float(Ncol))
            nc.vector.tensor_mul(tmp, mean, mean)
            nc.vector.tensor_add(tmp, tmp, var)
            nc.vector.tensor_scalar_mul(ccin_sb[:, 2 * h + 1:2 * h + 2], tmp,
                                        float(Ncol))
        USE_AG = False
        with tc.tile_pool(name="dram", bufs=1, space="DRAM") as dram:
            cc_in = dram.tile([AH, 8], f32, tag="ccin")
            if USE_AG:
                cc_out = dram.tile([AH * NCORE, 8], f32, tag="ccout")
                nc.gpsimd.dma_start(out=cc_in[:], in_=ccin_sb[:])
                nc.gpsimd.collective_compute(
                    "AllGather", ALU.bypass,
                    replica_groups=[list(range(NCORE))],
                    ins=[cc_in.opt()], outs=[cc_out.opt()])
                nc.gpsimd.dma_start(
                    out=ccg_sb[:],
                    in_=cc_out[:].rearrange("(r p) c -> p r c", r=NCORE))
            else:
                cc_out = dram.tile([AH, 8], f32, tag="ccout")
                nc.gpsimd.dma_start(out=cc_in[:], in_=ccin_sb[:])
                nc.gpsimd.collective_compute(
                    "AllReduce", ALU.add,
                    replica_groups=[list(range(NCORE))],
                    ins=[cc_in.opt()], outs=[cc_out.opt()])
                nc.gpsimd.dma_start(out=ccg_sb[:, 0, :], in_=cc_out[:])
        if USE_AG:
            # tree-reduce the 8 ranks' partials
            nc.vector.tensor_add(ccg_sb[:, 0:4, :], ccg_sb[:, 0:4, :],
                                 ccg_sb[:, 4:8, :])
            nc.vector.tensor_add(ccg_sb[:, 0:2, :], ccg_sb[:, 0:2, :],
                                 ccg_sb[:, 2:4, :])
            nc.vector.tensor_add(ccg_sb[:, 0:1, :], ccg_sb[:, 0:1, :],
                                 ccg_sb[:, 1:2, :])

        # Finalize: mean, rstd, -mean*rstd per half (ddof=1 over real T)
        Tf = float(T)
        for h in range(2):
            S1 = ccg_sb[:, 0, 2 * h:2 * h + 1]
            S2 = ccg_sb[:, 0, 2 * h + 1:2 * h + 2]
            mean = fin[:, h, 0:1]
            rstd = fin[:, h, 1:2]
            nb = fin[:, h, 2:3]
            tmp = fin[:, h, 3:4]
            nc.vector.tensor_scalar_mul(mean, S1, 1.0 / Tf)
            nc.vector.tensor_mul(tmp, S1, mean)
            nc.vector.tensor_sub(tmp, S2, tmp)
            nc.vector.tensor_scalar_mul(tmp, tmp, 1.0 / (Tf - 1.0))
            nc.scalar.activation(out=tmp, in_=tmp, func=AF.Sqrt, bias=eps_sb,
                                 scale=1.0)
            nc.vector.reciprocal(rstd, tmp)
            nc.vector.tensor_mul(nb, mean, rstd)
            nc.vector.tensor_scalar_mul(nb, nb, -1.0)

        # Tail: gate + weighted column-dot via PE (+ b2 via ones-row matmul)
        with tc.tile_pool(name="tails", bufs=3) as tails, \
                tc.tile_pool(name="psD", bufs=4, space="PSUM") as psD:
            for ci in range(nch_t):
                c0 = ci * TCH
                w = min(TCH, Ncol - c0)
                nsub = -(-w // 512)
                pss = [psD.tile([1, 512], f32, tag="d", name=f"psd{ci}_{k}")
                       for k in range(nsub)]
                y_ts = []
                for h in range(2):
                    s_t = tails.tile([AH, TCH], bf16, tag="s")
                    nc.scalar.activation(out=s_t[:, :w],
                                         in_=z_sb[:, h, c0:c0 + w],
                                         func=AF.Sigmoid,
                                         bias=fin[:, h, 2:3],
                                         scale=fin[:, h, 1:2])
                    y_t = tails.tile([AH, TCH], bf16, tag="y")
                    nc.vector.tensor_mul(y_t[:, :w], z_sb[:, h, c0:c0 + w],
                                         s_t[:, :w])
                    y_ts.append(y_t)
                for si in range(nsub):
                    s0 = si * 512
                    sw = min(512, w - s0)
                    nmm = 0
                    total = 2 + (2 if alpha_nz else 0) + (1 if b2_nz else 0)
                    for h in range(2):
                        nc.tensor.matmul(out=pss[si][:, :sw],
                                         lhsT=wdot_sb[:, h:h + 1],
                                         rhs=y_ts[h][:, s0:s0 + sw],
                                         start=(nmm == 0),
                                         stop=(nmm == total - 1))
                        nmm += 1
                    if alpha_nz:
                        for h in range(2):
                            nc.tensor.matmul(
                                out=pss[si][:, :sw],
                                lhsT=wdot_sb[:, 2 + h:3 + h],
                                rhs=z_sb[:, h, c0 + s0:c0 + s0 + sw],
                                start=False, stop=(nmm == total - 1))
                            nmm += 1
                    if b2_nz:
                        nc.tensor.matmul(out=pss[si][:, :sw],
                                         lhsT=b2_sb, rhs=ones_sb[:, :sw],
                                         start=False, stop=True)
                        nmm += 1
                    nc.scalar.copy(out=out_sb[:, c0 + s0:c0 + s0 + sw],
                                   in_=pss[si][:, :sw])
            nc.sync.dma_start(out=outd.ap(), in_=out_sb)

    nc.compile()
    return nc, in_maps, dict(T=T, idx_map=idx_map, valid=valid)


def _gather_output(meta, results):
    full = np.zeros((meta["T"], 1), np.float32)
    for c in range(NCORE):
        o = np.asarray(results[c]["out"], np.float32).reshape(-1)
        full[meta["idx_map"][c][meta["valid"][c]], 0] = o[meta["valid"][c]]
    return full


def _build_and_run(x, query, gather_idx, W1, b1, alpha, W2, b2):
    import os
    from concourse import bass_utils
    nc, in_maps, meta = _build(x, query, gather_idx, W1, b1, alpha, W2, b2)
    trace = bool(os.environ.get("DIN_TRACE"))
    res = bass_utils.run_bass_kernel_spmd(nc, in_maps,
                                          core_ids=list(range(NCORE)),
                                          trace=trace,
                                          trace_cores=list(range(NCORE))
                                          if trace else None)
    global LAST_EXEC_NS, LAST_RESULT
    LAST_EXEC_NS = res.exec_time_ns
    LAST_RESULT = res
    return _gather_output(meta, res.results)


def kernel(x, query, gather_idx, W1, b1, alpha, W2, b2):
    return _build_and_run(
        np.asarray(x, np.float32), np.asarray(query, np.float32),
        np.asarray(gather_idx), np.asarray(W1, np.float32),
        np.asarray(b1, np.float32), np.asarray(alpha, np.float32),
        np.asarray(W2, np.float32), np.asarray(b2, np.float32))


# revision 11
# speedup vs baseline: 1.3836x; 1.0455x over previous
"""DIN-style attention + Dice + MLP kernel for 8 trn2 NeuronCores.

Math (reference):
    q = query[gather_idx]                  # [T, 64]
    p = flat outer(x, q)                   # [T, 4096]
    h = [x, p, q]                          # [T, 4224]
    z = h @ W1 + b1                        # [T, 256]
    z = Dice(z)  (batch-global mean/var over T, ddof=1, sigmoid gate)
    out = z @ W2 + b2                      # [T, 1]

Key factorization: for t in group b (gather_idx[t] == b),
    z[t] = x_aug[t] @ D_b,   x_aug = [x, mask],
    D_b[j', a] = (j'<64): W1x[j',a] + sum_j query[b,j] W1p[j',j,a]
                 (j'=64): sum_j query[b,j] W1q[j,a] + b1[a]
so the [T,4096] outer-product features are never materialized; the dense
[T,4224]x[4224,256] matmul (137 GFLOP) becomes ~5 GFLOP of small matmuls.

Sharding: timesteps are grouped by gather value; the 512 groups are dealt
round-robin by descending size to 8 cores x 64 slots, so slot s has the same
padded width G_s on every core (one SPMD graph). Padded columns have x=0 and
mask=0 so their z is exactly 0 and global Dice sums (AllGathered across
cores, 4KB) stay exact with T hardcoded as the real count.
"""

import numpy as np
import ml_dtypes

NCORE = 8
LAST_EXEC_NS = None
LAST_RESULT = None


def _build(x, query, gather_idx, W1, b1, alpha, W2, b2):
    import concourse.bass as bass
    import concourse.tile as tile
    from concourse import bacc, mybir, bass_utils
    from contextlib import ExitStack

    f32 = mybir.dt.float32
    bf16 = mybir.dt.bfloat16
    AF = mybir.ActivationFunctionType
    ALU = mybir.AluOpType
    bf_np = ml_dtypes.bfloat16

    T, D = x.shape
    B = query.shape[0]
    A = W1.shape[1]
    EPS = 1e-9
    SLOTS = B // NCORE
    assert W1.shape[0] == D + D * D + D and B % NCORE == 0

    # ---- host-side sharding / layout ------------------------------------
    counts = np.bincount(gather_idx, minlength=B)
    order = np.argsort(-counts, kind="stable")  # groups by count desc
    Gs = []
    for s in range(SLOTS):
        m = int(counts[order[s * NCORE:(s + 1) * NCORE]].max())
        Gs.append(max(8, -(-m // 8) * 8))
    col_start = np.concatenate([[0], np.cumsum(Gs)]).astype(np.int64)
    Ncol = int(col_start[-1])
    assert max(Gs) <= 512, f"group too large: {max(Gs)}"

    # pack slots into PSUM-bank-sized column ranges (<=512 fp32)
    packs = []  # (slot_lo, slot_hi) half-open
    lo = 0
    while lo < SLOTS:
        hi = lo + 1
        while hi < SLOTS and col_start[hi + 1] - col_start[lo] <= 512:
            hi += 1
        packs.append((lo, hi))
        lo = hi
    NP = len(packs)

    sort_t = np.argsort(gather_idx, kind="stable")
    gstart = np.concatenate([[0], np.cumsum(counts)]).astype(np.int64)

    xT = np.ascontiguousarray(x.T.astype(np.float32))
    Xc = np.zeros((NCORE, D + 1, Ncol), np.float32)
    Qc = np.zeros((NCORE, D + 1, SLOTS), np.float32)
    idx_map = np.zeros((NCORE, Ncol), np.int64)
    valid = np.zeros((NCORE, Ncol), bool)
    for c in range(NCORE):
        for s in range(SLOTS):
            g = int(order[s * NCORE + c])
            n = int(counts[g])
            c0 = int(col_start[s])
            ts = sort_t[gstart[g]:gstart[g] + n]
            Xc[c, :D, c0:c0 + n] = xT[:, ts]
            Xc[c, D, c0:c0 + n] = 1.0
            idx_map[c, c0:c0 + n] = ts
            valid[c, c0:c0 + n] = True
            Qc[c, :D, s] = query[g]
            Qc[c, D, s] = 1.0
    Xc16 = np.ascontiguousarray(Xc.astype(bf_np))
    Qc16 = np.ascontiguousarray(Qc.astype(bf_np))

    W1x = W1[:D]
    W1p = W1[D:D + D * D].reshape(D, D, A)  # [i, j, a]
    W1q = W1[D + D * D:]
    Waug = np.zeros((D + 1, D + 1, A), np.float32)  # [j, i', a]
    Waug[:D, :D, :] = np.transpose(W1p, (1, 0, 2))
    Waug[:D, D, :] = W1q
    Waug[D, :D, :] = W1x
    Waug[D, D, :] = b1
    Waug16 = np.ascontiguousarray(Waug.astype(bf_np))

    al = float(np.asarray(alpha).reshape(-1)[0])
    alpha_nz = al != 0.0
    b2f = float(np.asarray(b2).reshape(-1)[0])
    b2_nz = b2f != 0.0
    w2v = np.asarray(W2, np.float32).reshape(-1)
    w_y = w2v * (1.0 - al)
    w_z = w2v * al
    AH = A // 2  # 128
    wdot = np.stack([w_y[:AH], w_y[AH:], w_z[:AH], w_z[AH:]], axis=1)
    wdot16 = np.ascontiguousarray(wdot.astype(bf_np))
    b2v = np.asarray([[b2f]]).astype(bf_np)

    in_maps = [
        {"xc": Xc16[c], "qc": Qc16[c], "waug": Waug16, "wdot": wdot16,
         "b2": b2v}
        for c in range(NCORE)
    ]

    # ---- device graph ----------------------------------------------------
    nc = bacc.Bacc("TRN2", target_bir_lowering=False, debug=False,
                   num_devices=NCORE)
    xd = nc.dram_tensor("xc", [D + 1, Ncol], bf16, kind="ExternalInput")
    qd = nc.dram_tensor("qc", [D + 1, SLOTS], bf16, kind="ExternalInput")
    wd = nc.dram_tensor("waug", [D + 1, D + 1, A], bf16, kind="ExternalInput")
    wdotd = nc.dram_tensor("wdot", [AH, 4], bf16, kind="ExternalInput")
    b2d = nc.dram_tensor("b2", [1, 1], bf16, kind="ExternalInput")
    outd = nc.dram_tensor("out", [1, Ncol], f32, kind="ExternalOutput")

    ABLK = 8          # a-columns per C-stage psum tile
    WCHUNK = 32       # a-columns per waug DMA chunk
    TCH = 1024        # tail sigmoid/mul chunk
    nch_t = -(-Ncol // TCH)

    with tile.TileContext(nc) as tc, ExitStack() as ctx:
        consts = ctx.enter_context(tc.tile_pool(name="consts", bufs=1))
        waug_sb = consts.tile([D + 1, D + 1, A], bf16, tag="waug")
        qc_sb = consts.tile([D + 1, SLOTS], bf16, tag="qc")
        x_sb = consts.tile([D + 1, Ncol], bf16, tag="x")
        wdot_sb = consts.tile([AH, 4], bf16, tag="wdot")
        b2_sb = consts.tile([1, 1], bf16, tag="b2")
        ones_sb = consts.tile([1, 512], bf16, tag="ones")
        eps_sb = consts.tile([AH, 1], f32, tag="eps")
        warm_sb = consts.tile([AH, 1], f32, tag="warm")
        dpp = consts.tile([D + 1, A, SLOTS], bf16, tag="dpp")
        z_sb = consts.tile([AH, 2, Ncol], bf16, tag="z")
        out_sb = consts.tile([1, Ncol], f32, tag="outsb")
        stats = consts.tile([AH, 2, NP, 6], f32, tag="stats")
        mv = consts.tile([AH, 2, 2], f32, tag="mv")
        ccin_sb = consts.tile([AH, 16], f32, tag="cc")
        ccg_sb = consts.tile([AH, NCORE, 16], f32, tag="ccg")
        fin = consts.tile([AH, 2, 4], f32, tag="fin")

        # input DMAs; waug chunked along a so the C-stage starts early
        nc.sync.dma_start(out=qc_sb, in_=qd.ap())
        for q0 in range(0, A, WCHUNK):
            nc.sync.dma_start(out=waug_sb[:, :, q0:q0 + WCHUNK],
                              in_=wd.ap()[:, :, q0:q0 + WCHUNK])
        nc.sync.dma_start(out=x_sb, in_=xd.ap())
        nc.sync.dma_start(out=wdot_sb, in_=wdotd.ap())
        nc.sync.dma_start(out=b2_sb, in_=b2d.ap())
        nc.vector.memset(eps_sb, EPS)
        nc.vector.memset(ones_sb, 1.0)
        nc.vector.memset(warm_sb, 0.0)
        # pre-load ACT tables off the critical path
        nc.scalar.activation(out=warm_sb, in_=warm_sb, func=AF.Sigmoid)
        nc.scalar.activation(out=warm_sb, in_=warm_sb, func=AF.Identity,
                             bias=0.0, scale=1.0)

        # C-stage: per-slot MLP matrices D_b, built as 256 small matmuls
        # out[i', b] = sum_j Waug[j, i', a] * q_aug[j, b], laid [i', a, b]
        with tc.tile_pool(name="psC", bufs=6, space="PSUM") as psC:
            for blk in range(A // ABLK):
                ps = psC.tile([D + 1, ABLK, SLOTS], f32, tag="c")
                for k in range(ABLK):
                    a = blk * ABLK + k
                    nc.tensor.matmul(out=ps[:, k, :], lhsT=waug_sb[:, :, a],
                                     rhs=qc_sb, start=True, stop=True)
                nc.scalar.copy(out=dpp[:, blk * ABLK:(blk + 1) * ABLK, :],
                               in_=ps)

        # Priming collective: absorbs CC-stack startup while compute runs.
        with tc.tile_pool(name="dramp", bufs=1, space="DRAM") as dramp:
            pr_in = dramp.tile([AH, 8], f32, tag="prin")
            pr_out = dramp.tile([AH * NCORE, 8], f32, tag="prout")
            pr_sb = consts.tile([AH, 8], f32, tag="prs")
            nc.vector.memset(pr_sb, 0.0)
            nc.gpsimd.dma_start(out=pr_in[:], in_=pr_sb)
            nc.gpsimd.collective_compute(
                "AllGather", ALU.bypass,
                replica_groups=[list(range(NCORE))],
                ins=[pr_in.opt()], outs=[pr_out.opt()])

        # Group stage: z^T[a, t] per slot; slots packed into 512-col psum
        # tiles. Half h=0 first so its stats collective overlaps h=1 compute.
        Tf = float(T)
        with tc.tile_pool(name="psG", bufs=6, space="PSUM") as psG, \
                tc.tile_pool(name="dram", bufs=1, space="DRAM") as dram:
            for h in range(2):
                for pi, (lo, hi) in enumerate(packs):
                    p0 = int(col_start[lo])
                    wsum = int(col_start[hi]) - p0
                    ps = psG.tile([AH, 512], f32, tag="g", name=f"g{h}_{pi}")
                    for s in range(lo, hi):
                        c0 = int(col_start[s]) - p0
                        w = Gs[s]
                        nc.tensor.matmul(
                            out=ps[:, c0:c0 + w],
                            lhsT=dpp[:, h * AH:(h + 1) * AH, s],
                            rhs=x_sb[:, p0 + c0:p0 + c0 + w],
                            start=True, stop=True)
                    nc.scalar.copy(out=z_sb[:, h, p0:p0 + wsum],
                                   in_=ps[:, :wsum])
                    nc.vector.bn_stats(out=stats[:, h, pi, :],
                                       in_=z_sb[:, h, p0:p0 + wsum])
                # stats for this half -> [S1, S2] partials -> AllGather
                nc.vector.bn_aggr(out=mv[:, h, :], in_=stats[:, h, :, :])
                mean = mv[:, h, 0:1]
                var = mv[:, h, 1:2]
                tmp = fin[:, h, 3:4]
                nc.vector.memset(ccin_sb[:, 8 * h + 2:8 * h + 8], 0.0)
                nc.vector.tensor_scalar_mul(
                    ccin_sb[:, 8 * h:8 * h + 1], mean, float(Ncol))
                nc.vector.tensor_mul(tmp, mean, mean)
                nc.vector.tensor_add(tmp, tmp, var)
                nc.vector.tensor_scalar_mul(
                    ccin_sb[:, 8 * h + 1:8 * h + 2], tmp, float(Ncol))
                cc_in = dram.tile([AH, 8], f32, tag=f"ccin{h}",
                                  name=f"ccin{h}")
                cc_out = dram.tile([AH * NCORE, 8], f32, tag=f"ccout{h}",
                                   name=f"ccout{h}")
                nc.gpsimd.dma_start(out=cc_in[:],
                                    in_=ccin_sb[:, 8 * h:8 * h + 8])
                nc.gpsimd.collective_compute(
                    "AllGather", ALU.bypass,
                    replica_groups=[list(range(NCORE))],
                    ins=[cc_in.opt()], outs=[cc_out.opt()])
                nc.gpsimd.dma_start(
                    out=ccg_sb[:, :, 8 * h:8 * h + 8].opt(),
                    in_=cc_out[:].rearrange("(r p) c -> p r c", r=NCORE))
                # tree-reduce the 8 ranks' partials, then mean/rstd/-mean*rstd
                cg = ccg_sb[:, :, 8 * h:8 * h + 8]
                nc.vector.tensor_add(cg[:, 0:4, :2], cg[:, 0:4, :2],
                                     cg[:, 4:8, :2])
                nc.vector.tensor_add(cg[:, 0:2, :2], cg[:, 0:2, :2],
                                     cg[:, 2:4, :2])
                nc.vector.tensor_add(cg[:, 0:1, :2], cg[:, 0:1, :2],
                                     cg[:, 1:2, :2])
                S1 = cg[:, 0, 0:1]
                S2 = cg[:, 0, 1:2]
                meanf = fin[:, h, 0:1]
                rstd = fin[:, h, 1:2]
                nb = fin[:, h, 2:3]
                tmpf = fin[:, h, 3:4]
                nc.vector.tensor_scalar_mul(meanf, S1, 1.0 / Tf)
                nc.vector.tensor_mul(tmpf, S1, meanf)
                nc.vector.tensor_sub(tmpf, S2, tmpf)
                nc.vector.tensor_scalar_mul(tmpf, tmpf, 1.0 / (Tf - 1.0))
                nc.scalar.activation(out=tmpf, in_=tmpf, func=AF.Sqrt,
                                     bias=eps_sb, scale=1.0)
                nc.vector.reciprocal(rstd, tmpf)
                nc.vector.tensor_mul(nb, meanf, rstd)
                nc.vector.tensor_scalar_mul(nb, nb, -1.0)

        # Tail: gate + weighted column-dot via PE; h=0 runs while h=1's
        # AllGather is still in flight. Dot psum goes straight to DRAM.
        with tc.tile_pool(name="tails", bufs=3) as tails, \
                tc.tile_pool(name="psD", bufs=4, space="PSUM") as psD:
            n_h_mm = 2 if alpha_nz else 1
            total_mm = 2 * n_h_mm + (1 if b2_nz else 0)
            for ci in range(nch_t):
                c0 = ci * TCH
                w = min(TCH, Ncol - c0)
                nsub = -(-w // 512)
                pss = [psD.tile([1, 512], f32, tag="d", name=f"psd{ci}_{k}")
                       for k in range(nsub)]
                for h in range(2):
                    s_t = tails.tile([AH, TCH], bf16, tag="s",
                                     name=f"s{ci}_{h}")
                    nc.scalar.activation(out=s_t[:, :w],
                                         in_=z_sb[:, h, c0:c0 + w],
                                         func=AF.Sigmoid,
                                         bias=fin[:, h, 2:3],
                                         scale=fin[:, h, 1:2])
                    y_t = tails.tile([AH, TCH], bf16, tag="y",
                                     name=f"y{ci}_{h}")
                    nc.vector.tensor_mul(y_t[:, :w], z_sb[:, h, c0:c0 + w],
                                         s_t[:, :w])
                    for si in range(nsub):
                        s0 = si * 512
                        sw = min(512, w - s0)
                        nmm = h * n_h_mm
                        nc.tensor.matmul(out=pss[si][:, :sw],
                                         lhsT=wdot_sb[:, h:h + 1],
                                         rhs=y_t[:, s0:s0 + sw],
                                         start=(nmm == 0),
                                         stop=(nmm == total_mm - 1))
                        if alpha_nz:
                            nmm += 1
                            nc.tensor.matmul(
                                out=pss[si][:, :sw],
                                lhsT=wdot_sb[:, 2 + h:3 + h],
                                rhs=z_sb[:, h, c0 + s0:c0 + s0 + sw],
                                start=False, stop=(nmm == total_mm - 1))
                for si in range(nsub):
                    s0 = si * 512
                    sw = min(512, w - s0)
                    if b2_nz:
                        nc.tensor.matmul(out=pss[si][:, :sw],
                                         lhsT=b2_sb, rhs=ones_sb[:, :sw],
                                         start=False, stop=True)
                    nc.any.tensor_copy(out=out_sb[:, c0 + s0:c0 + s0 + sw],
                                       in_=pss[si][:, :sw])
            nc.sync.dma_start(out=outd.ap(), in_=out_sb)

    nc.compile()
    return nc, in_maps, dict(T=T, idx_map=idx_map, valid=valid)


def _gather_output(meta, results):
    full = np.zeros((meta["T"], 1), np.float32)
    for c in range(NCORE):
        o = np.asarray(results[c]["out"], np.float32).reshape(-1)
        full[meta["idx_map"][c][meta["valid"][c]], 0] = o[meta["valid"][c]]
    return full


def _build_and_run(x, query, gather_idx, W1, b1, alpha, W2, b2):
    import os
    from concourse import bass_utils
    nc, in_maps, meta = _build(x, query, gather_idx, W1, b1, alpha, W2, b2)
    trace = bool(os.environ.get("DIN_TRACE"))
    res = bass_utils.run_bass_kernel_spmd(nc, in_maps,
                                          core_ids=list(range(NCORE)),
                                          trace=trace,
                                          trace_cores=list(range(NCORE))
                                          if trace else None)
    global LAST_EXEC_NS, LAST_RESULT
    LAST_EXEC_NS = res.exec_time_ns
    LAST_RESULT = res
    return _gather_output(meta, res.results)


def kernel(x, query, gather_idx, W1, b1, alpha, W2, b2):
    return _build_and_run(
        np.asarray(x, np.float32), np.asarray(query, np.float32),
        np.asarray(gather_idx), np.asarray(W1, np.float32),
        np.asarray(b1, np.float32), np.asarray(alpha, np.float32),
        np.asarray(W2, np.float32), np.asarray(b2, np.float32))


# revision 12
# speedup vs baseline: 1.6460x; 1.1896x over previous
"""DIN-style attention + Dice + MLP kernel for 8 trn2 NeuronCores.

Math (reference):
    q = query[gather_idx]                  # [T, 64]
    p = flat outer(x, q)                   # [T, 4096]
    h = [x, p, q]                          # [T, 4224]
    z = h @ W1 + b1                        # [T, 256]
    z = Dice(z)  (batch-global mean/var over T, ddof=1, sigmoid gate)
    out = z @ W2 + b2                      # [T, 1]

Key factorization: for t in group b (gather_idx[t] == b),
    z[t] = x_aug[t] @ D_b,   x_aug = [x, mask],
    D_b[j', a] = (j'<64): W1x[j',a] + sum_j query[b,j] W1p[j',j,a]
                 (j'=64): sum_j query[b,j] W1q[j,a] + b1[a]
so the [T,4096] outer-product features are never materialized; the dense
[T,4224]x[4224,256] matmul (137 GFLOP) becomes ~5 GFLOP of small matmuls.

Sharding: timesteps are grouped by gather value; the 512 groups are dealt
round-robin by descending size to 8 cores x 64 slots, so slot s has the same
padded width G_s on every core (one SPMD graph). Padded columns have x=0 and
mask=0 so their z is exactly 0 and global Dice sums (AllGathered across
cores, 4KB) stay exact with T hardcoded as the real count.
"""

import numpy as np
import ml_dtypes

NCORE = 8
LAST_EXEC_NS = None
LAST_RESULT = None


def _build(x, query, gather_idx, W1, b1, alpha, W2, b2):
    import concourse.bass as bass
    import concourse.tile as tile
    from concourse import bacc, mybir, bass_utils
    from contextlib import ExitStack

    f32 = mybir.dt.float32
    bf16 = mybir.dt.bfloat16
    AF = mybir.ActivationFunctionType
    ALU = mybir.AluOpType
    bf_np = ml_dtypes.bfloat16

    T, D = x.shape
    B = query.shape[0]
    A = W1.shape[1]
    EPS = 1e-9
    SLOTS = B // NCORE
    assert W1.shape[0] == D + D * D + D and B % NCORE == 0

    # ---- host-side sharding / layout ------------------------------------
    counts = np.bincount(gather_idx, minlength=B)
    order = np.argsort(-counts, kind="stable")  # groups by count desc
    Gs = []
    for s in range(SLOTS):
        m = int(counts[order[s * NCORE:(s + 1) * NCORE]].max())
        Gs.append(max(8, -(-m // 8) * 8))
    col_start = np.concatenate([[0], np.cumsum(Gs)]).astype(np.int64)
    Ncol = int(col_start[-1])
    assert max(Gs) <= 512, f"group too large: {max(Gs)}"

    # pack slots into PSUM-bank-sized column ranges (<=512 fp32)
    packs = []  # (slot_lo, slot_hi) half-open
    lo = 0
    while lo < SLOTS:
        hi = lo + 1
        while hi < SLOTS and col_start[hi + 1] - col_start[lo] <= 512:
            hi += 1
        packs.append((lo, hi))
        lo = hi
    NP = len(packs)

    sort_t = np.argsort(gather_idx, kind="stable")
    gstart = np.concatenate([[0], np.cumsum(counts)]).astype(np.int64)

    xT = np.ascontiguousarray(x.T.astype(np.float32))
    Xc = np.zeros((NCORE, D + 1, Ncol), np.float32)
    Qc = np.zeros((NCORE, D + 1, SLOTS), np.float32)
    idx_map = np.zeros((NCORE, Ncol), np.int64)
    valid = np.zeros((NCORE, Ncol), bool)
    for c in range(NCORE):
        for s in range(SLOTS):
            g = int(order[s * NCORE + c])
            n = int(counts[g])
            c0 = int(col_start[s])
            ts = sort_t[gstart[g]:gstart[g] + n]
            Xc[c, :D, c0:c0 + n] = xT[:, ts]
            Xc[c, D, c0:c0 + n] = 1.0
            idx_map[c, c0:c0 + n] = ts
            valid[c, c0:c0 + n] = True
            Qc[c, :D, s] = query[g]
            Qc[c, D, s] = 1.0
    Xc16 = np.ascontiguousarray(Xc.astype(bf_np))
    Qc16 = np.ascontiguousarray(Qc.astype(bf_np))

    W1x = W1[:D]
    W1p = W1[D:D + D * D].reshape(D, D, A)  # [i, j, a]
    W1q = W1[D + D * D:]
    Waug = np.zeros((D + 1, D + 1, A), np.float32)  # [j, i', a]
    Waug[:D, :D, :] = np.transpose(W1p, (1, 0, 2))
    Waug[:D, D, :] = W1q
    Waug[D, :D, :] = W1x
    Waug[D, D, :] = b1
    Waug16 = np.ascontiguousarray(Waug.transpose(0, 2, 1).astype(bf_np))

    al = float(np.asarray(alpha).reshape(-1)[0])
    alpha_nz = al != 0.0
    b2f = float(np.asarray(b2).reshape(-1)[0])
    b2_nz = b2f != 0.0
    w2v = np.asarray(W2, np.float32).reshape(-1)
    w_y = w2v * (1.0 - al)
    w_z = w2v * al
    AH = A // 2  # 128
    wdot = np.stack([w_y[:AH], w_y[AH:], w_z[:AH], w_z[AH:]], axis=1)
    wdot16 = np.ascontiguousarray(wdot.astype(bf_np))
    b2v = np.asarray([[b2f]]).astype(bf_np)

    in_maps = [
        {"xc": Xc16[c], "qc": Qc16[c], "waug": Waug16, "wdot": wdot16,
         "b2": b2v}
        for c in range(NCORE)
    ]

    # ---- device graph ----------------------------------------------------
    nc = bacc.Bacc("TRN2", target_bir_lowering=False, debug=False,
                   num_devices=NCORE)
    xd = nc.dram_tensor("xc", [D + 1, Ncol], bf16, kind="ExternalInput")
    qd = nc.dram_tensor("qc", [D + 1, SLOTS], bf16, kind="ExternalInput")
    wd = nc.dram_tensor("waug", [D + 1, A, D + 1], bf16, kind="ExternalInput")
    wdotd = nc.dram_tensor("wdot", [AH, 4], bf16, kind="ExternalInput")
    b2d = nc.dram_tensor("b2", [1, 1], bf16, kind="ExternalInput")
    outd = nc.dram_tensor("out", [1, Ncol], f32, kind="ExternalOutput")

    ABLK = 8          # a-columns per C-stage psum tile
    WCHUNK = 32       # a-columns per waug DMA chunk
    TCH = 1024        # tail sigmoid/mul chunk
    nch_t = -(-Ncol // TCH)

    with tile.TileContext(nc) as tc, ExitStack() as ctx:
        consts = ctx.enter_context(tc.tile_pool(name="consts", bufs=1))
        waug_sb = consts.tile([D + 1, A, D + 1], bf16, tag="waug")
        qc_sb = consts.tile([D + 1, SLOTS], bf16, tag="qc")
        x_sb = consts.tile([D + 1, Ncol], bf16, tag="x")
        wdot_sb = consts.tile([AH, 4], bf16, tag="wdot")
        b2_sb = consts.tile([1, 1], bf16, tag="b2")
        ones_sb = consts.tile([1, 512], bf16, tag="ones")
        eps_sb = consts.tile([AH, 1], f32, tag="eps")
        warm_sb = consts.tile([AH, 1], f32, tag="warm")
        dpp = consts.tile([D + 1, A, SLOTS], bf16, tag="dpp")
        z_sb = consts.tile([AH, 2, Ncol], bf16, tag="z")
        out_sb = consts.tile([1, Ncol], f32, tag="outsb")
        stats = consts.tile([AH, 2, NP, 6], f32, tag="stats")
        mv = consts.tile([AH, 2, 2], f32, tag="mv")
        ccin_sb = consts.tile([AH, 16], f32, tag="cc")
        ccg_sb = consts.tile([AH, NCORE, 16], f32, tag="ccg")
        fin = consts.tile([AH, 2, 4], f32, tag="fin")

        # input DMAs; waug chunked along a so the C-stage starts early
        nc.sync.dma_start(out=qc_sb, in_=qd.ap())
        for q0 in range(0, A, WCHUNK):
            nc.sync.dma_start(out=waug_sb[:, q0:q0 + WCHUNK, :],
                              in_=wd.ap()[:, q0:q0 + WCHUNK, :])
        nc.sync.dma_start(out=x_sb, in_=xd.ap())
        nc.sync.dma_start(out=wdot_sb, in_=wdotd.ap())
        nc.sync.dma_start(out=b2_sb, in_=b2d.ap())
        nc.vector.memset(eps_sb, EPS)
        nc.vector.memset(ones_sb, 1.0)
        nc.vector.memset(warm_sb, 0.0)
        # pre-load ACT tables off the critical path
        nc.scalar.activation(out=warm_sb, in_=warm_sb, func=AF.Sigmoid)
        nc.scalar.activation(out=warm_sb, in_=warm_sb, func=AF.Identity,
                             bias=0.0, scale=1.0)

        # C-stage: per-slot MLP matrices D_b, built as 256 small matmuls
        # out[i', b] = sum_j Waug[j, i', a] * q_aug[j, b], laid [i', a, b]
        with tc.tile_pool(name="psC", bufs=6, space="PSUM") as psC:
            for blk in range(A // ABLK):
                ps = psC.tile([D + 1, ABLK, SLOTS], f32, tag="c")
                for k in range(ABLK):
                    a = blk * ABLK + k
                    nc.tensor.matmul(out=ps[:, k, :], lhsT=waug_sb[:, a, :],
                                     rhs=qc_sb, start=True, stop=True)
                nc.scalar.copy(out=dpp[:, blk * ABLK:(blk + 1) * ABLK, :],
                               in_=ps)

        # Group stage: z^T[a, t] per slot; slots packed into 512-col psum
        # tiles. Half h=0 first so its stats collective overlaps h=1 compute.
        Tf = float(T)
        with tc.tile_pool(name="psG", bufs=6, space="PSUM") as psG:
            for h in range(2):
                for pi, (lo, hi) in enumerate(packs):
                    p0 = int(col_start[lo])
                    wsum = int(col_start[hi]) - p0
                    ps = psG.tile([AH, 512], f32, tag="g", name=f"g{h}_{pi}")
                    for s in range(lo, hi):
                        c0 = int(col_start[s]) - p0
                        w = Gs[s]
                        nc.tensor.matmul(
                            out=ps[:, c0:c0 + w],
                            lhsT=dpp[:, h * AH:(h + 1) * AH, s],
                            rhs=x_sb[:, p0 + c0:p0 + c0 + w],
                            start=True, stop=True)
                    nc.scalar.copy(out=z_sb[:, h, p0:p0 + wsum],
                                   in_=ps[:, :wsum])
                    nc.vector.bn_stats(out=stats[:, h, pi, :],
                                       in_=z_sb[:, h, p0:p0 + wsum])
        # Stats: both halves -> [S1,S2] x2 -> one 4KB AllGather -> finalize
        for h in range(2):
            nc.vector.bn_aggr(out=mv[:, h, :], in_=stats[:, h, :, :])
            mean = mv[:, h, 0:1]
            var = mv[:, h, 1:2]
            tmp = fin[:, h, 3:4]
            nc.vector.tensor_scalar_mul(
                ccin_sb[:, 2 * h:2 * h + 1], mean, float(Ncol))
            nc.vector.tensor_mul(tmp, mean, mean)
            nc.vector.tensor_add(tmp, tmp, var)
            nc.vector.tensor_scalar_mul(
                ccin_sb[:, 2 * h + 1:2 * h + 2], tmp, float(Ncol))
        nc.vector.memset(ccin_sb[:, 4:8], 0.0)
        with tc.tile_pool(name="dram", bufs=1, space="DRAM") as dram:
            cc_in = dram.tile([AH, 8], f32, tag="ccin")
            cc_out = dram.tile([AH * NCORE, 8], f32, tag="ccout")
            nc.gpsimd.dma_start(out=cc_in[:], in_=ccin_sb[:, 0:8])
            nc.gpsimd.collective_compute(
                "AllGather", ALU.bypass,
                replica_groups=[list(range(NCORE))],
                ins=[cc_in.opt()], outs=[cc_out.opt()])
            nc.gpsimd.dma_start(
                out=ccg_sb[:, :, 0:8].opt(),
                in_=cc_out[:].rearrange("(r p) c -> p r c", r=NCORE))
        cg = ccg_sb[:, :, 0:8]
        nc.vector.tensor_add(cg[:, 0:4, :4], cg[:, 0:4, :4], cg[:, 4:8, :4])
        nc.vector.tensor_add(cg[:, 0:2, :4], cg[:, 0:2, :4], cg[:, 2:4, :4])
        nc.vector.tensor_add(cg[:, 0:1, :4], cg[:, 0:1, :4], cg[:, 1:2, :4])
        for h in range(2):
            S1 = cg[:, 0, 2 * h:2 * h + 1]
            S2 = cg[:, 0, 2 * h + 1:2 * h + 2]
            meanf = fin[:, h, 0:1]
            rstd = fin[:, h, 1:2]
            nb = fin[:, h, 2:3]
            tmpf = fin[:, h, 3:4]
            nc.vector.tensor_scalar_mul(meanf, S1, 1.0 / Tf)
            nc.vector.tensor_mul(tmpf, S1, meanf)
            nc.vector.tensor_sub(tmpf, S2, tmpf)
            nc.vector.tensor_scalar_mul(tmpf, tmpf, 1.0 / (Tf - 1.0))
            nc.scalar.activation(out=tmpf, in_=tmpf, func=AF.Sqrt,
                                 bias=eps_sb, scale=1.0)
            nc.vector.reciprocal(rstd, tmpf)
            nc.vector.tensor_mul(nb, meanf, rstd)
            nc.vector.tensor_scalar_mul(nb, nb, -1.0)

        # Tail: gate + weighted column-dot via PE; h=0 runs while h=1's
        # AllGather is still in flight. Dot psum goes straight to DRAM.
        with tc.tile_pool(name="tails", bufs=3) as tails, \
                tc.tile_pool(name="psD", bufs=4, space="PSUM") as psD:
            n_h_mm = 2 if alpha_nz else 1
            total_mm = 2 * n_h_mm + (1 if b2_nz else 0)
            for ci in range(nch_t):
                c0 = ci * TCH
                w = min(TCH, Ncol - c0)
                nsub = -(-w // 512)
                pss = [psD.tile([1, 512], f32, tag="d", name=f"psd{ci}_{k}")
                       for k in range(nsub)]
                for h in range(2):
                    s_t = tails.tile([AH, TCH], bf16, tag="s",
                                     name=f"s{ci}_{h}")
                    nc.scalar.activation(out=s_t[:, :w],
                                         in_=z_sb[:, h, c0:c0 + w],
                                         func=AF.Sigmoid,
                                         bias=fin[:, h, 2:3],
                                         scale=fin[:, h, 1:2])
                    y_t = tails.tile([AH, TCH], bf16, tag="y",
                                     name=f"y{ci}_{h}")
                    nc.vector.tensor_mul(y_t[:, :w], z_sb[:, h, c0:c0 + w],
                                         s_t[:, :w])
                    for si in range(nsub):
                        s0 = si * 512
                        sw = min(512, w - s0)
                        nmm = h * n_h_mm
                        nc.tensor.matmul(out=pss[si][:, :sw],
                                         lhsT=wdot_sb[:, h:h + 1],
                                         rhs=y_t[:, s0:s0 + sw],
                                         start=(nmm == 0),
                                         stop=(nmm == total_mm - 1))
                        if alpha_nz:
                            nmm += 1
                            nc.tensor.matmul(
                                out=pss[si][:, :sw],
                                lhsT=wdot_sb[:, 2 + h:3 + h],
                                rhs=z_sb[:, h, c0 + s0:c0 + s0 + sw],
                                start=False, stop=(nmm == total_mm - 1))
                for si in range(nsub):
                    s0 = si * 512
                    sw = min(512, w - s0)
                    if b2_nz:
                        nc.tensor.matmul(out=pss[si][:, :sw],
                                         lhsT=b2_sb, rhs=ones_sb[:, :sw],
                                         start=False, stop=True)
                    nc.any.tensor_copy(out=out_sb[:, c0 + s0:c0 + s0 + sw],
                                       in_=pss[si][:, :sw])
            nc.sync.dma_start(out=outd.ap(), in_=out_sb)

    nc.compile()
    return nc, in_maps, dict(T=T, idx_map=idx_map, valid=valid)


def _gather_output(meta, results):
    full = np.zeros((meta["T"], 1), np.float32)
    for c in range(NCORE):
        o = np.asarray(results[c]["out"], np.float32).reshape(-1)
        full[meta["idx_map"][c][meta["valid"][c]], 0] = o[meta["valid"][c]]
    return full


def _build_and_run(x, query, gather_idx, W1, b1, alpha, W2, b2):
    import os
    from concourse import bass_utils
    nc, in_maps, meta = _build(x, query, gather_idx, W1, b1, alpha, W2, b2)
    trace = bool(os.environ.get("DIN_TRACE"))
    res = bass_utils.run_bass_kernel_spmd(nc, in_maps,
                                          core_ids=list(range(NCORE)),
                                          trace=trace,
                                          trace_cores=list(range(NCORE))
                                          if trace else None)
    global LAST_EXEC_NS, LAST_RESULT
    LAST_EXEC_NS = res.exec_time_ns
    LAST_RESULT = res
    return _gather_output(meta, res.results)


def kernel(x, query, gather_idx, W1, b1, alpha, W2, b2):
    return _build_and_run(
        np.asarray(x, np.float32), np.asarray(query, np.float32),
        np.asarray(gather_idx), np.asarray(W1, np.float32),
        np.asarray(b1, np.float32), np.asarray(alpha, np.float32),
        np.asarray(W2, np.float32), np.asarray(b2, np.float32))
